# revision 28
# baseline (speedup 1.0000x reference)
"""nn_GateModLinear on 8 trn2 NeuronCores.

z[b,:] = gW[b,:] * sum_m pW[b,m] * (Ws[m] @ x[b]) + gb[b,:] * (pb @ bs)[b,:]
out = ELU(LayerNorm(z))

Sharding: data-parallel over batch (512 rows/core), Ws replicated.
Per core: fold pW into x per expert (host-precomputed xs[m] = pW[:,m]*x,
0.05% of FLOPs), then accumulate all (m, j) into PSUM on the PE:
  Wx[b,i] = sum_{m,j} xs[m,b,j] * Ws[m,i,j]
bf16 matmuls (rel-err budget 2e-2), fp32 PSUM accumulate.

The bias path zb = gb*(pb@bs) is fully precomputed on host (0.003% of
the FLOPs) and streamed as bf16 — no fp32 bias matmuls on the PE.
ws chunks alternate between the two HWDGE rings (sync/scalar) so the
SDMA round-robin gives ws 2/3 of HBM bandwidth, matching consumption.
Epilogue is all DVE+ACT (GpSimd ucode ops stall the DVE via SBUF
contention — avoid): drain = acc*gw (PSUM 1x), += zb (bf16 2x), LN
stats per half, fused 7-op Newton rsqrt, and a one-op ELU:
  ELU(y) = min(exp(y)-1, max(y,0))
with exp on ACT and the min on DVE scalar_tensor_tensor.
"""

import numpy as np
import ml_dtypes

B, M, DI, DO = 4096, 8, 2048, 2048
NCORES = 8
BS = B // NCORES  # 512 batch rows per core
LN_EPS = 1e-5
P = 128
JC = DI // P      # 16 contraction chunks of 128
BC = BS // P      # 4 batch chunks of 128
NIH = 2           # output-dim halves
IH = DO // NIH    # 1024
NQ = IH // 512    # 2 psum tiles of 512 per half

BF16 = ml_dtypes.bfloat16

_cache = {}


def _build():
    from contextlib import ExitStack
    import concourse.bacc as bacc
    import concourse.tile as tile
    from concourse import mybir

    f32 = mybir.dt.float32
    bf16 = mybir.dt.bfloat16
    i32 = mybir.dt.int32
    AF = mybir.ActivationFunctionType
    ALU = mybir.AluOpType

    nc = bacc.Bacc("TRN2", target_bir_lowering=False, debug=False, num_devices=1)
    xs_d = nc.dram_tensor("xs", [M, DI, BS], bf16, kind="ExternalInput")
    ws_d = nc.dram_tensor("wsT", [M, DI, DO], bf16, kind="ExternalInput")
    zb_d = nc.dram_tensor("zb", [BS, DO], bf16, kind="ExternalInput")
    gw_d = nc.dram_tensor("gw", [BS, DO], bf16, kind="ExternalInput")
    out_d = nc.dram_tensor("out", [BS, DO], bf16, kind="ExternalOutput")

    with ExitStack() as ctx:
        tc = ctx.enter_context(tile.TileContext(nc))
        singles = ctx.enter_context(tc.tile_pool(name="singles", bufs=1))
        ws_pool = ctx.enter_context(tc.tile_pool(name="ws", bufs=18))
        xs_pool = ctx.enter_context(tc.tile_pool(name="xs", bufs=10))
        e_pool = ctx.enter_context(tc.tile_pool(name="elu", bufs=3))
        sm_pool = ctx.enter_context(tc.tile_pool(name="small", bufs=4))
        ps_pool = ctx.enter_context(tc.tile_pool(name="ps", bufs=8, space="PSUM"))

        phases = [(ih, m) for ih in range(NIH) for m in range(M)]

        def load(idx):
            ih, m = phases[idx]
            # jc-chunk splits; phase 0 leads with small chunks so the
            # first matmuls' operands land as early as possible.
            if idx == 0:
                xsplit = [("x0", 2), ("x0", 2), ("xs", 4), ("xs", 4), ("xs", 4)]
                wsplit = [("w0", 1)] * 4 + [("ws", 2)] * 6
            else:
                xsplit = [("xs", 4)] * 4
                wsplit = [("ws", 2)] * 8
            xsrc = xs_d.ap()[m].rearrange("(jc jp) b -> jp jc b", jp=P)
            xmap = []
            off = 0
            for h, (tg, nj) in enumerate(xsplit):
                t = xs_pool.tile([P, nj, BS], bf16, tag=tg,
                                 bufs=2 if tg == "x0" else None,
                                 name=f"xs_{ih}_{m}_{h}")
                # the very first xs chunk rides the scalar HWDGE queue so
                # it lands in parallel with ws chunk 0 on the sync queue.
                eng = nc.scalar if (idx == 0 and h == 0) else nc.gpsimd
                eng.dma_start(out=t, in_=xsrc[:, off:off + nj, :])
                for j in range(nj):
                    xmap.append((t, j))
                off += nj
            wsrc = ws_d.ap()[m].rearrange("(jc jp) i -> jp jc i", jp=P)
            wmap = []
            off = 0
            for h, (tg, nj) in enumerate(wsplit):
                t = ws_pool.tile([P, nj, IH], bf16, tag=tg,
                                 bufs=4 if tg == "w0" else None,
                                 name=f"ws_{ih}_{m}_{h}")
                # alternate the two HWDGE rings: SDMA round-robins rings
                # fairly, so splitting ws across both gives it 2/3 of the
                # HBM bandwidth vs 1/3 for xs — matching consumption rates.
                weng = nc.sync if h % 2 == 0 else nc.scalar
                weng.dma_start(
                    out=t,
                    in_=wsrc[:, off:off + nj, ih * IH:(ih + 1) * IH],
                )
                for j in range(nj):
                    wmap.append((t, j))
                off += nj
            return xmap, wmap

        # ---- PE warm-up: short N=128 matmuls with no DMA deps keep the
        # PE activity monitor busy (clock at 2.4 GHz when the stream
        # starts) and bridge the first-chunk DMA latency ----
        wl = singles.tile([P, P], bf16)
        nc.vector.memset(wl, 1.0)
        wr = singles.tile([P, P], bf16)
        nc.vector.memset(wr, 0.5)
        wp = ps_pool.tile([P, 512], f32, tag="acc", name="warm")
        for _ in range(34):
            nc.tensor.matmul(wp[:, 0:P], wl, wr, start=True, stop=True)

        # prefetch phases 0 and 1 up front (ws on sync+scalar, xs gpsimd)
        pending = [load(0), load(1)]

        # zb/gw ride the sync HWDGE ring behind later phases' ws chunks:
        # ring FIFO order paces their transfers after the phase-3..6 ws
        # data, so they never compete with the phase-0/1 operand stream.
        zb = singles.tile([P, BC, DO], bf16)
        gw = singles.tile([P, BC, DO], bf16)
        zb_src = zb_d.ap().rearrange("(bc p) i -> p bc i", p=P)
        gw_src = gw_d.ap().rearrange("(bc p) i -> p bc i", p=P)

        z = singles.tile([P, BC, DO], bf16)
        stats = singles.tile([P, BC, 4, 6], f32)

        out_ap = out_d.ap().rearrange("(bc p) i -> p bc i", p=P)

        def drain_mul(ih, acc, bc, q):
            i0 = ih * IH + q * 512
            # z = acc * gw  (DVE 1x, PSUM source)
            nc.vector.tensor_mul(z[:, bc, i0:i0 + 512], acc[bc][q],
                                 gw[:, bc, i0:i0 + 512])

        def drain_finish_half(ih, bc):
            i0 = ih * IH
            zs = z[:, bc, i0:i0 + IH]
            # z += zb  (DVE 2x, all bf16 SBUF), then LN partial stats
            # (bn_stats free dim is HW-capped at 512 -> two chunk calls)
            nc.vector.tensor_add(zs, zs, zb[:, bc, i0:i0 + IH])
            for q in range(NQ):
                nc.vector.bn_stats(out=stats[:, bc, 2 * ih + q, :],
                                   in_=zs[:, q * 512:(q + 1) * 512])

        def drain_finish_q(ih, bc, q):
            i0 = ih * IH + q * 512
            zs = z[:, bc, i0:i0 + 512]
            nc.vector.tensor_add(zs, zs, zb[:, bc, i0:i0 + 512])
            nc.vector.bn_stats(out=stats[:, bc, 2 * ih + q, :], in_=zs)

        def ln_stats(bc):
            # LayerNorm scale for batch chunk bc: aggr + fast rsqrt.
            # Pure-DVE chain with no cross-engine waits.
            mv = sm_pool.tile([P, 2], f32, tag="mv", name=f"mv_{bc}")
            nc.vector.bn_aggr(out=mv, in_=stats[:, bc])
            # rstd = 1/sqrt(var+eps) via bitcast seed + 1 Newton step on
            # DVE (the ACT table never switches sets). The seed constant
            # is adjusted to read vh = (var+eps)/2 directly:
            #   bits(1/sqrt(2*vh)) ~ 0x5ef759df - (bits(vh) >> 1)
            vh = sm_pool.tile([P, 1], f32, tag="vh", name=f"vh_{bc}")
            nc.vector.tensor_scalar(vh, mv[:, 1:2], 0.5, 0.5 * LN_EPS,
                                    op0=ALU.mult, op1=ALU.add)
            rstd = sm_pool.tile([P, 1], f32, tag="rstd", name=f"rstd_{bc}")
            nc.vector.tensor_scalar(
                rstd.bitcast(i32), vh.bitcast(i32), 1, -1,
                op0=ALU.logical_shift_right, op1=ALU.bitwise_xor)
            nc.vector.tensor_scalar_add(rstd.bitcast(i32), rstd.bitcast(i32),
                                        0x5ef759e0)
            # y *= 1.5 - vh*y*y
            t1 = sm_pool.tile([P, 1], f32, tag="t1", name=f"t1_{bc}")
            nc.vector.tensor_mul(t1, rstd, rstd)
            nc.vector.tensor_mul(t1, t1, vh)
            nc.vector.tensor_scalar(t1, t1, -1.0, -1.5,
                                    op0=ALU.mult, op1=ALU.subtract)
            nc.vector.tensor_mul(rstd, rstd, t1)
            nmr = sm_pool.tile([P, 1], f32, tag="nmr", name=f"nmr_{bc}")
            nc.vector.scalar_tensor_tensor(nmr, mv[:, 0:1], -1.0, rstd,
                                           op0=ALU.mult, op1=ALU.mult)
            return rstd, nmr

        # per output half: y = rstd*z + nmr;
        # ELU(y) = min(exp(y)-1, max(y,0))
        def elu_act(bc, rstd, nmr):
            # ACT-side ELU work + rel store (the final add is offloaded
            # to the DMA datapath: store rel = max(y,0) now; a deferred
            # SWDGE store of min(exp(y)-1, 0) accum-adds onto it — both
            # ride the single SWDGE ring, which drains FIFO per engine).
            ets = []
            for h in range(2):
                hs = slice(h * (DO // 2), (h + 1) * (DO // 2))
                rh = z[:, bc, hs]
                rel = e_pool.tile([P, DO // 2], bf16, tag="rel",
                                  name=f"rel_{bc}_{h}")
                nc.scalar.activation(out=rel, in_=rh, func=AF.Relu,
                                     bias=nmr, scale=rstd)
                nc.gpsimd.dma_start(out=out_ap[:, bc, hs], in_=rel)
                et = e_pool.tile([P, DO // 2], bf16, tag="et", bufs=4,
                                 name=f"et_{bc}_{h}")
                nc.scalar.activation(out=et, in_=rh, func=AF.Exp,
                                     bias=nmr, scale=rstd)
                ets.append(et)
            return ets

        def elu_fix(bc, ets):
            # ACT-dependent DVE fixups + accum stores, emitted one batch
            # chunk LATER than elu_act so these semaphore-waiting DVE ops
            # never head-of-line block the next chunk's drain chain.
            for h, et in enumerate(ets):
                hs = slice(h * (DO // 2), (h + 1) * (DO // 2))
                nc.vector.tensor_scalar(et, et, -1.0, 0.0,
                                        op0=ALU.add, op1=ALU.min)
                nc.gpsimd.dma_start(out=out_ap[:, bc, hs], in_=et,
                                    accum_op=ALU.add)

        def elu_tail(bc, rstd, nmr, pend=None):
            # final chunk: whole chain on DVE/ACT, pipelined at 512-wide
            # quarters so the last store issues as early as possible.
            # The previous chunk's deferred fixups slot in between the
            # two halves: by then their Exp inputs are ready (no DVE
            # wait) and their accum stores still beat the final store.
            for h in range(2):
                hs = slice(h * (DO // 2), (h + 1) * (DO // 2))
                rh = z[:, bc, hs]
                rel = e_pool.tile([P, DO // 2], bf16, tag="rel",
                                  name=f"rel_{bc}_{h}")
                nc.vector.tensor_scalar(rel, rh, rstd, nmr,
                                        op0=ALU.mult, op1=ALU.add)
                nc.vector.tensor_scalar_max(rel, rel, 0.0)
                for qt in range(2):
                    qs = slice(h * (DO // 2) + qt * 512,
                               h * (DO // 2) + (qt + 1) * 512)
                    et = e_pool.tile([P, 512], bf16, tag="etq", bufs=4,
                                     name=f"et_{bc}_{h}_{qt}")
                    nc.scalar.activation(out=et, in_=z[:, bc, qs],
                                         func=AF.Exp,
                                         bias=nmr, scale=rstd)
                    ot = e_pool.tile([P, 512], bf16, tag="otq", bufs=4,
                                     name=f"ot_{bc}_{h}_{qt}")
                    nc.vector.scalar_tensor_tensor(
                        ot, et, -1.0, rel[:, qt * 512:(qt + 1) * 512],
                        op0=ALU.add, op1=ALU.min)
                    nc.sync.dma_start(out=out_ap[:, bc, qs], in_=ot)
                if h == 0 and pend is not None:
                    elu_fix(bc - 1, pend)
                    pend = None

        # ---- main accumulation ----
        for idx, (ih, m) in enumerate(phases):
            xmap, wmap = pending.pop(0)
            if idx + 2 < len(phases):
                pending.append(load(idx + 2))
            if 1 <= idx <= BC:
                bc = idx - 1
                nc.sync.dma_start(out=zb[:, bc, :], in_=zb_src[:, bc, :])
                nc.sync.dma_start(out=gw[:, bc, :], in_=gw_src[:, bc, :])
            if m == 0:
                acc = [[ps_pool.tile([P, 512], f32, tag="acc",
                                     name=f"acc_{ih}_{bc}_{q}")
                        for q in range(NQ)] for bc in range(BC)]
            last = (m == M - 1)
            if last:
                # bc-major, q-major per-(bc,q) 16-MM units so each PSUM
                # bank's drain chain starts as early as possible
                units = [(bc, q) for bc in range(BC) for q in range(NQ)]
                pend_ets = {}
                for bc, q in units:
                    tailbc = (bc == BC - 1)
                    for jc in range(JC):
                        xt, xj = xmap[jc]
                        w, wj = wmap[jc]
                        nc.tensor.matmul(
                            acc[bc][q],
                            xt[:, xj, bc * P:(bc + 1) * P],
                            w[:, wj, q * 512:(q + 1) * 512],
                            start=False,
                            stop=(jc == JC - 1),
                        )
                    drain_mul(ih, acc, bc, q)
                    if tailbc and ih == NIH - 1:
                        drain_finish_q(ih, bc, q)
                    elif q == NQ - 1:
                        drain_finish_half(ih, bc)
                    if ih == NIH - 1 and q == NQ - 1:
                        rstd, nmr = ln_stats(bc)
                        if not tailbc:
                            pend_ets[bc] = elu_act(bc, rstd, nmr)
                            if bc - 1 in pend_ets:
                                elu_fix(bc - 1, pend_ets.pop(bc - 1))
                        else:
                            elu_tail(bc, rstd, nmr,
                                     pend=pend_ets.pop(bc - 1, None))
            else:
                for jc in range(JC):
                    xt, xj = xmap[jc]
                    w, wj = wmap[jc]
                    for bc in range(BC):
                        for q in range(NQ):
                            nc.tensor.matmul(
                                acc[bc][q],
                                xt[:, xj, bc * P:(bc + 1) * P],
                                w[:, wj, q * 512:(q + 1) * 512],
                                start=(m == 0 and jc == 0),
                                stop=False,
                            )

    nc.compile()
    return nc


def _prep_inputs(x, Ws, bs, pW, pb, gW, gb):
    x = np.asarray(x, np.float32)
    pW = np.asarray(pW, np.float32)
    # xs[m, j, b] = pW[b, m] * x[b, j], bf16, per-core column slices
    xT = np.ascontiguousarray(x.T)                        # [DI, B]
    wsT = np.ascontiguousarray(
        np.asarray(Ws, np.float32).transpose(0, 2, 1)
    ).astype(BF16)                                        # [M, DI, DO]
    # bias path entirely on host: zb = gb * (pb @ bs)    [B, DO]
    zb = (np.asarray(gb, np.float32)
          * (np.asarray(pb, np.float32) @ np.asarray(bs, np.float32)))
    zb16 = zb.astype(BF16)
    gW16 = np.asarray(gW, np.float32).astype(BF16)
    in_maps = []
    for c in range(NCORES):
        sl = slice(c * BS, (c + 1) * BS)
        xs = (pW[sl].T[:, None, :] * xT[None, :, sl]).astype(BF16)
        in_maps.append({
            "xs": np.ascontiguousarray(xs),               # [M, DI, BS]
            "wsT": wsT,
            "zb": np.ascontiguousarray(zb16[sl]),
            "gw": np.ascontiguousarray(gW16[sl]),
        })
    return in_maps


def kernel(x, Ws, bs, pW, pb, gW, gb, _trace=False, _tmpdir=None):
    from concourse import bass_utils

    if "nc" not in _cache:
        _cache["nc"] = _build()
    nc = _cache["nc"]
    in_maps = _prep_inputs(x, Ws, bs, pW, pb, gW, gb)
    res = bass_utils.run_bass_kernel_spmd(
        nc, in_maps, core_ids=list(range(NCORES)),
        trace=_trace, tmpdir=_tmpdir,
    )
    _cache["last_result"] = res
    out = np.concatenate([res.results[c]["out"] for c in range(NCORES)], axis=0)
    return np.asarray(out, dtype=np.float32)


# revision 30
# speedup vs baseline: 1.0111x; 1.0111x over previous
"""nn_GateModLinear on 8 trn2 NeuronCores.

z[b,:] = gW[b,:] * sum_m pW[b,m] * (Ws[m] @ x[b]) + gb[b,:] * (pb @ bs)[b,:]
out = ELU(LayerNorm(z))

Sharding: data-parallel over batch (512 rows/core), Ws replicated.
Per core: fold pW into x per expert (host-precomputed xs[m] = pW[:,m]*x,
0.05% of FLOPs), then accumulate all (m, j) into PSUM on the PE:
  Wx[b,i] = sum_{m,j} xs[m,b,j] * Ws[m,i,j]
bf16 matmuls (rel-err budget 2e-2), fp32 PSUM accumulate.

The bias path zb = gb*(pb@bs) is fully precomputed on host (0.003% of
the FLOPs) and streamed as bf16 — no fp32 bias matmuls on the PE.
ws chunks alternate between the two HWDGE rings (sync/scalar) so the
SDMA round-robin gives ws 2/3 of HBM bandwidth, matching consumption.
Epilogue is all DVE+ACT (GpSimd ucode ops stall the DVE via SBUF
contention — avoid): drain = acc*gw (PSUM 1x), += zb (bf16 2x), LN
stats per half, fused 7-op Newton rsqrt, and a one-op ELU:
  ELU(y) = min(exp(y)-1, max(y,0))
with exp on ACT and the min on DVE scalar_tensor_tensor.
"""

import numpy as np
import ml_dtypes

B, M, DI, DO = 4096, 8, 2048, 2048
NCORES = 8
BS = B // NCORES  # 512 batch rows per core
LN_EPS = 1e-5
P = 128
JC = DI // P      # 16 contraction chunks of 128
BC = BS // P      # 4 batch chunks of 128
NIH = 2           # output-dim halves
IH = DO // NIH    # 1024
NQ = IH // 512    # 2 psum tiles of 512 per half

BF16 = ml_dtypes.bfloat16

_cache = {}


def _build():
    from contextlib import ExitStack
    import concourse.bacc as bacc
    import concourse.tile as tile
    from concourse import mybir

    f32 = mybir.dt.float32
    bf16 = mybir.dt.bfloat16
    i32 = mybir.dt.int32
    AF = mybir.ActivationFunctionType
    ALU = mybir.AluOpType

    nc = bacc.Bacc("TRN2", target_bir_lowering=False, debug=False, num_devices=1)
    xs_d = nc.dram_tensor("xs", [M, DI, BS], bf16, kind="ExternalInput")
    ws_d = nc.dram_tensor("wsT", [M, DI, DO], bf16, kind="ExternalInput")
    zb_d = nc.dram_tensor("zb", [BS, DO], bf16, kind="ExternalInput")
    gw_d = nc.dram_tensor("gw", [BS, DO], bf16, kind="ExternalInput")
    out_d = nc.dram_tensor("out", [BS, DO], bf16, kind="ExternalOutput")

    with ExitStack() as ctx:
        tc = ctx.enter_context(tile.TileContext(nc))
        singles = ctx.enter_context(tc.tile_pool(name="singles", bufs=1))
        ws_pool = ctx.enter_context(tc.tile_pool(name="ws", bufs=18))
        xs_pool = ctx.enter_context(tc.tile_pool(name="xs", bufs=10))
        e_pool = ctx.enter_context(tc.tile_pool(name="elu", bufs=3))
        sm_pool = ctx.enter_context(tc.tile_pool(name="small", bufs=4))
        ps_pool = ctx.enter_context(tc.tile_pool(name="ps", bufs=8, space="PSUM"))

        phases = [(ih, m) for ih in range(NIH) for m in range(M)]

        def load(idx):
            ih, m = phases[idx]
            # jc-chunk splits; phase 0 leads with small chunks so the
            # first matmuls' operands land as early as possible.
            if idx == 0:
                xsplit = [("x0", 2), ("x0", 2), ("xs", 4), ("xs", 4), ("xs", 4)]
                wsplit = [("w0", 1), ("w0", 1)] + [("ws", 2)] * 7
            else:
                xsplit = [("xs", 4)] * 4
                wsplit = [("ws", 2)] * 8
            xsrc = xs_d.ap()[m].rearrange("(jc jp) b -> jp jc b", jp=P)
            xmap = []
            off = 0
            for h, (tg, nj) in enumerate(xsplit):
                t = xs_pool.tile([P, nj, BS], bf16, tag=tg,
                                 bufs=2 if tg == "x0" else None,
                                 name=f"xs_{ih}_{m}_{h}")
                # the very first xs chunk rides the scalar HWDGE queue so
                # it lands in parallel with ws chunk 0 on the sync queue.
                eng = nc.scalar if (idx == 0 and h == 0) else nc.gpsimd
                eng.dma_start(out=t, in_=xsrc[:, off:off + nj, :])
                for j in range(nj):
                    xmap.append((t, j))
                off += nj
            wsrc = ws_d.ap()[m].rearrange("(jc jp) i -> jp jc i", jp=P)
            wmap = []
            off = 0
            for h, (tg, nj) in enumerate(wsplit):
                t = ws_pool.tile([P, nj, IH], bf16, tag=tg,
                                 bufs=2 if tg == "w0" else None,
                                 name=f"ws_{ih}_{m}_{h}")
                # alternate the two HWDGE rings: SDMA round-robins rings
                # fairly, so splitting ws across both gives it 2/3 of the
                # HBM bandwidth vs 1/3 for xs — matching consumption rates.
                weng = nc.sync if h % 2 == 0 else nc.scalar
                weng.dma_start(
                    out=t,
                    in_=wsrc[:, off:off + nj, ih * IH:(ih + 1) * IH],
                )
                for j in range(nj):
                    wmap.append((t, j))
                off += nj
            return xmap, wmap

        # ---- PE warm-up: short N=128 matmuls with no DMA deps keep the
        # PE activity monitor busy (clock at 2.4 GHz when the stream
        # starts) and bridge the first-chunk DMA latency ----
        wl = singles.tile([P, P], bf16)
        nc.vector.memset(wl, 1.0)
        wr = singles.tile([P, P], bf16)
        nc.vector.memset(wr, 0.5)
        wp = ps_pool.tile([P, 512], f32, tag="acc", name="warm")
        for _ in range(34):
            nc.tensor.matmul(wp[:, 0:P], wl, wr, start=True, stop=True)

        # prefetch phases 0 and 1 up front (ws on sync+scalar, xs gpsimd)
        pending = [load(0), load(1)]

        # zb/gw ride the sync HWDGE ring behind later phases' ws chunks:
        # ring FIFO order paces their transfers after the phase-3..6 ws
        # data, so they never compete with the phase-0/1 operand stream.
        zb = singles.tile([P, BC, DO], bf16)
        gw = singles.tile([P, BC, DO], bf16)
        zb_src = zb_d.ap().rearrange("(bc p) i -> p bc i", p=P)
        gw_src = gw_d.ap().rearrange("(bc p) i -> p bc i", p=P)

        z = singles.tile([P, BC, DO], bf16)
        stats = singles.tile([P, BC, 4, 6], f32)

        out_ap = out_d.ap().rearrange("(bc p) i -> p bc i", p=P)

        def drain_mul(ih, acc, bc, q):
            i0 = ih * IH + q * 512
            # z = acc * gw  (DVE 1x, PSUM source)
            nc.vector.tensor_mul(z[:, bc, i0:i0 + 512], acc[bc][q],
                                 gw[:, bc, i0:i0 + 512])

        def drain_finish_half(ih, bc):
            i0 = ih * IH
            zs = z[:, bc, i0:i0 + IH]
            # z += zb  (DVE 2x, all bf16 SBUF), then LN partial stats
            # (bn_stats free dim is HW-capped at 512 -> two chunk calls)
            nc.vector.tensor_add(zs, zs, zb[:, bc, i0:i0 + IH])
            for q in range(NQ):
                nc.vector.bn_stats(out=stats[:, bc, 2 * ih + q, :],
                                   in_=zs[:, q * 512:(q + 1) * 512])

        def drain_finish_q(ih, bc, q):
            i0 = ih * IH + q * 512
            zs = z[:, bc, i0:i0 + 512]
            nc.vector.tensor_add(zs, zs, zb[:, bc, i0:i0 + 512])
            nc.vector.bn_stats(out=stats[:, bc, 2 * ih + q, :], in_=zs)

        def ln_stats(bc):
            # LayerNorm scale for batch chunk bc: aggr + fast rsqrt.
            # Pure-DVE chain with no cross-engine waits.
            mv = sm_pool.tile([P, 2], f32, tag="mv", name=f"mv_{bc}")
            nc.vector.bn_aggr(out=mv, in_=stats[:, bc])
            # rstd = 1/sqrt(var+eps) via bitcast seed + 1 Newton step on
            # DVE (the ACT table never switches sets). The seed constant
            # is adjusted to read vh = (var+eps)/2 directly:
            #   bits(1/sqrt(2*vh)) ~ 0x5ef759df - (bits(vh) >> 1)
            vh = sm_pool.tile([P, 1], f32, tag="vh", name=f"vh_{bc}")
            nc.vector.tensor_scalar(vh, mv[:, 1:2], 0.5, 0.5 * LN_EPS,
                                    op0=ALU.mult, op1=ALU.add)
            rstd = sm_pool.tile([P, 1], f32, tag="rstd", name=f"rstd_{bc}")
            nc.vector.tensor_scalar(
                rstd.bitcast(i32), vh.bitcast(i32), 1, -1,
                op0=ALU.logical_shift_right, op1=ALU.bitwise_xor)
            nc.vector.tensor_scalar_add(rstd.bitcast(i32), rstd.bitcast(i32),
                                        0x5ef759e0)
            # y *= 1.5 - vh*y*y
            t1 = sm_pool.tile([P, 1], f32, tag="t1", name=f"t1_{bc}")
            nc.vector.tensor_mul(t1, rstd, rstd)
            nc.vector.tensor_mul(t1, t1, vh)
            nc.vector.tensor_scalar(t1, t1, -1.0, -1.5,
                                    op0=ALU.mult, op1=ALU.subtract)
            nc.vector.tensor_mul(rstd, rstd, t1)
            nmr = sm_pool.tile([P, 1], f32, tag="nmr", name=f"nmr_{bc}")
            nc.vector.scalar_tensor_tensor(nmr, mv[:, 0:1], -1.0, rstd,
                                           op0=ALU.mult, op1=ALU.mult)
            return rstd, nmr

        # per output half: y = rstd*z + nmr;
        # ELU(y) = min(exp(y)-1, max(y,0))
        def elu_act(bc, rstd, nmr):
            # ACT-side ELU work + rel store (the final add is offloaded
            # to the DMA datapath: store rel = max(y,0) now; a deferred
            # SWDGE store of min(exp(y)-1, 0) accum-adds onto it — both
            # ride the single SWDGE ring, which drains FIFO per engine).
            ets = []
            for h in range(2):
                hs = slice(h * (DO // 2), (h + 1) * (DO // 2))
                rh = z[:, bc, hs]
                rel = e_pool.tile([P, DO // 2], bf16, tag="rel",
                                  name=f"rel_{bc}_{h}")
                nc.scalar.activation(out=rel, in_=rh, func=AF.Relu,
                                     bias=nmr, scale=rstd)
                nc.gpsimd.dma_start(out=out_ap[:, bc, hs], in_=rel)
                et = e_pool.tile([P, DO // 2], bf16, tag="et", bufs=4,
                                 name=f"et_{bc}_{h}")
                nc.scalar.activation(out=et, in_=rh, func=AF.Exp,
                                     bias=nmr, scale=rstd)
                ets.append(et)
            return ets

        def elu_fix(bc, ets):
            # ACT-dependent DVE fixups + accum stores, emitted one batch
            # chunk LATER than elu_act so these semaphore-waiting DVE ops
            # never head-of-line block the next chunk's drain chain.
            for h, et in enumerate(ets):
                hs = slice(h * (DO // 2), (h + 1) * (DO // 2))
                nc.vector.tensor_scalar(et, et, -1.0, 0.0,
                                        op0=ALU.add, op1=ALU.min)
                nc.gpsimd.dma_start(out=out_ap[:, bc, hs], in_=et,
                                    accum_op=ALU.add)

        def elu_tail(bc, rstd, nmr, pend=None):
            # final chunk: whole chain on DVE/ACT, pipelined at 512-wide
            # quarters so the last store issues as early as possible.
            # The previous chunk's deferred fixups slot in between the
            # two halves: by then their Exp inputs are ready (no DVE
            # wait) and their accum stores still beat the final store.
            for h in range(2):
                hs = slice(h * (DO // 2), (h + 1) * (DO // 2))
                rh = z[:, bc, hs]
                rel = e_pool.tile([P, DO // 2], bf16, tag="rel",
                                  name=f"rel_{bc}_{h}")
                nc.vector.tensor_scalar(rel, rh, rstd, nmr,
                                        op0=ALU.mult, op1=ALU.add)
                nc.vector.tensor_scalar_max(rel, rel, 0.0)
                for qt in range(2):
                    qs = slice(h * (DO // 2) + qt * 512,
                               h * (DO // 2) + (qt + 1) * 512)
                    et = e_pool.tile([P, 512], bf16, tag="etq", bufs=4,
                                     name=f"et_{bc}_{h}_{qt}")
                    nc.scalar.activation(out=et, in_=z[:, bc, qs],
                                         func=AF.Exp,
                                         bias=nmr, scale=rstd)
                    ot = e_pool.tile([P, 512], bf16, tag="otq", bufs=4,
                                     name=f"ot_{bc}_{h}_{qt}")
                    nc.vector.scalar_tensor_tensor(
                        ot, et, -1.0, rel[:, qt * 512:(qt + 1) * 512],
                        op0=ALU.add, op1=ALU.min)
                    nc.sync.dma_start(out=out_ap[:, bc, qs], in_=ot)
                if h == 0 and pend is not None:
                    elu_fix(bc - 1, pend)
                    pend = None

        # ---- main accumulation ----
        for idx, (ih, m) in enumerate(phases):
            xmap, wmap = pending.pop(0)
            if idx + 2 < len(phases):
                pending.append(load(idx + 2))
            if 1 <= idx <= BC:
                bc = idx - 1
                nc.sync.dma_start(out=zb[:, bc, :], in_=zb_src[:, bc, :])
                nc.sync.dma_start(out=gw[:, bc, :], in_=gw_src[:, bc, :])
            if m == 0:
                acc = [[ps_pool.tile([P, 512], f32, tag="acc",
                                     name=f"acc_{ih}_{bc}_{q}")
                        for q in range(NQ)] for bc in range(BC)]
            last = (m == M - 1)
            if last:
                # bc-major, q-major per-(bc,q) 16-MM units so each PSUM
                # bank's drain chain starts as early as possible
                units = [(bc, q) for bc in range(BC) for q in range(NQ)]
                pend_ets = {}
                for bc, q in units:
                    tailbc = (bc == BC - 1)
                    for jc in range(JC):
                        xt, xj = xmap[jc]
                        w, wj = wmap[jc]
                        nc.tensor.matmul(
                            acc[bc][q],
                            xt[:, xj, bc * P:(bc + 1) * P],
                            w[:, wj, q * 512:(q + 1) * 512],
                            start=False,
                            stop=(jc == JC - 1),
                        )
                    drain_mul(ih, acc, bc, q)
                    if tailbc and ih == NIH - 1:
                        drain_finish_q(ih, bc, q)
                    elif q == NQ - 1:
                        drain_finish_half(ih, bc)
                    if ih == NIH - 1 and q == NQ - 1:
                        rstd, nmr = ln_stats(bc)
                        if not tailbc:
                            pend_ets[bc] = elu_act(bc, rstd, nmr)
                            if bc - 1 in pend_ets:
                                elu_fix(bc - 1, pend_ets.pop(bc - 1))
                        else:
                            elu_tail(bc, rstd, nmr,
                                     pend=pend_ets.pop(bc - 1, None))
            else:
                for jc in range(JC):
                    xt, xj = xmap[jc]
                    w, wj = wmap[jc]
                    for bc in range(BC):
                        for q in range(NQ):
                            nc.tensor.matmul(
                                acc[bc][q],
                                xt[:, xj, bc * P:(bc + 1) * P],
                                w[:, wj, q * 512:(q + 1) * 512],
                                start=(m == 0 and jc == 0),
                                stop=False,
                            )

    nc.compile()
    return nc


def _prep_inputs(x, Ws, bs, pW, pb, gW, gb):
    x = np.asarray(x, np.float32)
    pW = np.asarray(pW, np.float32)
    # xs[m, j, b] = pW[b, m] * x[b, j], bf16, per-core column slices
    xT = np.ascontiguousarray(x.T)                        # [DI, B]
    wsT = np.ascontiguousarray(
        np.asarray(Ws, np.float32).transpose(0, 2, 1)
    ).astype(BF16)                                        # [M, DI, DO]
    # bias path entirely on host: zb = gb * (pb @ bs)    [B, DO]
    zb = (np.asarray(gb, np.float32)
          * (np.asarray(pb, np.float32) @ np.asarray(bs, np.float32)))
    zb16 = zb.astype(BF16)
    gW16 = np.asarray(gW, np.float32).astype(BF16)
    in_maps = []
    for c in range(NCORES):
        sl = slice(c * BS, (c + 1) * BS)
        xs = (pW[sl].T[:, None, :] * xT[None, :, sl]).astype(BF16)
        in_maps.append({
            "xs": np.ascontiguousarray(xs),               # [M, DI, BS]
            "wsT": wsT,
            "zb": np.ascontiguousarray(zb16[sl]),
            "gw": np.ascontiguousarray(gW16[sl]),
        })
    return in_maps


def kernel(x, Ws, bs, pW, pb, gW, gb, _trace=False, _tmpdir=None):
    from concourse import bass_utils

    if "nc" not in _cache:
        _cache["nc"] = _build()
    nc = _cache["nc"]
    in_maps = _prep_inputs(x, Ws, bs, pW, pb, gW, gb)
    res = bass_utils.run_bass_kernel_spmd(
        nc, in_maps, core_ids=list(range(NCORES)),
        trace=_trace, tmpdir=_tmpdir,
    )
    _cache["last_result"] = res
    out = np.concatenate([res.results[c]["out"] for c in range(NCORES)], axis=0)
    return np.asarray(out, dtype=np.float32)
